# revision 1
# baseline (speedup 1.0000x reference)
import sys

for _p in ("/opt/trn_rl_repo",):
    if _p not in sys.path:
        sys.path.insert(0, _p)

import numpy as np
import ml_dtypes

import concourse.bass as bass
import concourse.bacc as bacc
import concourse.tile as tile
import concourse.mybir as mybir
from concourse import bass_utils

F32 = mybir.dt.float32
BF16 = mybir.dt.bfloat16
FP8 = mybir.dt.float8e4

NP_BF16 = ml_dtypes.bfloat16
NP_FP8 = ml_dtypes.float8_e4m3

EMBED = 512
MID = 512
FINAL = 1024
GLIMPSES = 2
NOBJ = 2048
NREL = 32768
NCORES = 8
RCH = NREL // NCORES          # 4096 relations per core
NRC = RCH // 512              # 8 relation chunks of 512 per core
NRB = RCH // 128              # 32 relation blocks of 128 per core
CSCALE = 65536.0              # fp8 range scaling for recip in W

# Entity-window structure for the one-time aggregation A[e~, r].
# 151 entities are host-packed into 2 windows: window 0 holds 128 entities
# (W pieces 128 wide, full-M matmuls into psA0), window 1 holds 23 entities
# (W pieces 32 wide, matmuls into psA1[0:32]). Objects are sorted into the
# windows and padded to whole 128-object blocks. All matmuls run fp8
# DoubleRow over block pairs (no tile_position, walrus-legal).
WIN_BLOCKS = [14, 3]
WIN_CAP = [128 * b for b in WIN_BLOCKS]
WIN_ENTS = [128, 23]
WIN_WIDTH = [128, 32]
NBLK = sum(WIN_BLOCKS)        # 17 object blocks = 2176 padded objects
WCOL = [0, 14 * 128]          # wW column offset per window
NWCOL = 14 * 128 + 3 * 32     # total wW columns
NE = 160                      # padded entity count

VSCALE = 1024.0               # fp8 range scaling for abc tables

# wpack (bf16) column offsets — small bf16 weights coalesced into one tile,
# DMAed in two pieces: "early" (needed for qw0/A/v) and "late"
OBJT = 0                      # 4 * NE
WQT0 = OBJT + 4 * NE          # 4 * MID (glimpse-0 WqT)
IDENT = WQT0 + 4 * MID        # 128
ONESC = IDENT + 128           # 1
NWE = ONESC + 1               # end of early piece
WQT1 = NWE                    # 4 * MID (glimpse-1 WqT)
OTAB_HI = WQT1 + 4 * MID      # EMBED
OTAB_LO = OTAB_HI + EMBED     # EMBED (rows 0:32)
FC2TC = OTAB_LO + EMBED       # 4 * FINAL (bf16 fc2 weights)
WFOLD = FC2TC + 4 * FINAL     # 4 * MID (bf16 Wa0.T @ Wq1.T, k-blocks)
CNT2 = WFOLD + 4 * MID        # 2
NW = CNT2 + 2

# fpack (f32) column offsets
WAT = 0                       # GLIMPSES * 4 * MID
FC1T = WAT + GLIMPSES * 4 * MID      # 4 * MID
BAT = FC1T + 4 * MID          # GLIMPSES * 4
FC1BT = BAT + GLIMPSES * 4    # 4
ONESF = FC1BT + 4             # 1
NF = ONESF + 1

# rowpack (bf16) [1, x]: bqrow 0:1024, ones 1024:1152
RONES = GLIMPSES * MID
NR = RONES + 128

_CACHE = {}


def _wn(v, g):
    return (v * (g / np.linalg.norm(v.astype(np.float64)))).astype(np.float32)


def _build():
    """Builds the Bass program once. Returns (nc, input tensor names)."""
    nc = bacc.Bacc(
        "TRN2",
        target_bir_lowering=False,
        debug=False,
        enable_asserts=False,
        num_devices=NCORES,
    )

    # ---- DRAM I/O -------------------------------------------------------
    d_g = nc.dram_tensor("g", [NRC, 128, NBLK * 512], FP8,
                         kind="ExternalInput")
    d_W = nc.dram_tensor("W", [128, NWCOL], FP8, kind="ExternalInput")
    d_ohtdr = nc.dram_tensor("ohtdr", [77, 2 * RCH], FP8, kind="ExternalInput")
    d_abcdr = nc.dram_tensor("abcdr", [77, GLIMPSES * 1024], FP8,
                             kind="ExternalInput")
    d_wpack = nc.dram_tensor("wpack", [128, NW], BF16, kind="ExternalInput")
    d_fpack = nc.dram_tensor("fpack", [128, NF], F32, kind="ExternalInput")
    d_rowpack = nc.dram_tensor("rowpack", [1, NR], BF16, kind="ExternalInput")
    d_fc2b = nc.dram_tensor("fc2b", [1, FINAL], F32, kind="ExternalInput")
    d_out = nc.dram_tensor("out", [1, FINAL], F32, kind="ExternalOutput")

    with tile.TileContext(nc) as tc:
        _emit(nc, tc, locals())

    nc.compile()
    in_names = ["g", "W", "ohtdr", "abcdr", "wpack", "fpack", "rowpack", "fc2b"]
    return nc, in_names


def _emit(nc, tc, d):
    AT = mybir.ActivationFunctionType
    OP = mybir.AluOpType
    rg = [list(range(NCORES))]

    with (
        tc.tile_pool(name="persist", bufs=1) as pp,
        tc.tile_pool(name="work", bufs=4) as wp,
        tc.tile_pool(name="gpool", bufs=6) as gp,
        tc.tile_pool(name="pa0", bufs=1, space="PSUM") as pa0,
        tc.tile_pool(name="pvv", bufs=2, space="PSUM") as pvv,
        tc.tile_pool(name="pbh", bufs=2, space="PSUM") as pbhp,
        tc.tile_pool(name="pbl", bufs=2, space="PSUM") as pblp,
        tc.tile_pool(name="paux", bufs=1, space="PSUM") as paux,
        tc.tile_pool(name="dram", bufs=1, space="DRAM") as dp,
    ):
        # ---- coalesced loads --------------------------------------------
        # The first graph chunk on each HWDGE ring goes out FIRST so the
        # aggregation pipeline starts as early as possible; the small weight
        # packs follow (nothing on the critical path needs them sooner).
        g8rc = []
        for rc in range(2):
            t = gp.tile([128, NBLK * 512], FP8, name=f"g8rc{rc}", tag="g8")
            eng = nc.sync if rc % 2 == 0 else nc.scalar
            eng.dma_start(t[:], d["d_g"][rc])
            g8rc.append(t)
        wW = pp.tile([128, NWCOL], FP8, name="wW", tag="wW")
        nc.sync.dma_start(wW[:], d["d_W"][:, :])
        rpk = pp.tile([1, NR], BF16, name="rpk", tag="rpk")
        nc.sync.dma_start(rpk[:], d["d_rowpack"][:, :])
        wpk = pp.tile([128, NW], BF16, name="wpk", tag="wpk")
        nc.scalar.dma_start(wpk[:, 0:NWE], d["d_wpack"][:, 0:NWE])

        # rest of the graph stream (ring-buffered, alternating HWDGE rings)
        ohtdr = pp.tile([77, 2 * RCH], FP8, name="ohtdr", tag="ohtdr")
        abcdr = pp.tile([77, GLIMPSES * 1024], FP8, name="abcdr", tag="abcdr")
        for rc in range(2, NRC):
            t = gp.tile([128, NBLK * 512], FP8, name=f"g8rc{rc}", tag="g8")
            eng = nc.sync if rc % 2 == 0 else nc.scalar
            eng.dma_start(t[:], d["d_g"][rc])
            g8rc.append(t)
            if rc == 2:
                nc.sync.dma_start(ohtdr[:], d["d_ohtdr"][:, :])
                nc.sync.dma_start(abcdr[:], d["d_abcdr"][:, :])

        nc.scalar.dma_start(wpk[:, NWE:NW], d["d_wpack"][:, NWE:NW])
        fpk = pp.tile([128, NF], F32, name="fpk", tag="fpk")
        nc.scalar.dma_start(fpk[:], d["d_fpack"][:, :])
        fc2b = pp.tile([1, FINAL], F32, name="fc2b", tag="fc2b")
        nc.scalar.dma_start(fc2b[:], d["d_fc2b"][:, :])

        # ---- views into the packs ---------------------------------------
        objT = wpk[:, OBJT:OBJT + 4 * NE]
        wqTg = [wpk[:, WQT0:WQT0 + 4 * MID], wpk[:, WQT1:WQT1 + 4 * MID]]
        objtab_hi = wpk[:, OTAB_HI:OTAB_HI + EMBED]
        objtab_lo = wpk[0:32, OTAB_LO:OTAB_LO + EMBED]
        ident = wpk[:, IDENT:IDENT + 128]
        cnt2 = wpk[:, CNT2:CNT2 + 2]
        onesc = wpk[:, ONESC:ONESC + 1]
        fc2T = wpk[:, FC2TC:FC2TC + 4 * FINAL]
        wfold = wpk[:, WFOLD:WFOLD + 4 * MID]
        bqrow = rpk[0:1, 0:GLIMPSES * MID]
        ones = rpk[0:1, RONES:RONES + 128]
        waT = fpk[:, WAT:WAT + GLIMPSES * 4 * MID]
        fc1T = fpk[:, FC1T:FC1T + 4 * MID]
        baT = fpk[:, BAT:BAT + GLIMPSES * 4]
        fc1bT = fpk[:, FC1BT:FC1BT + 4]
        onesf = fpk[:, ONESF:ONESF + 1]

        # ---- sgq0 = cnt @ obj_tab (column sums of q0), partition layout
        sgq0_ps = paux.tile([128, 4], F32, name="sgq0_ps", tag="aux")
        for kt in range(4):
            nc.tensor.matmul(sgq0_ps[:, kt:kt + 1],
                             objtab_hi[:, kt * 128:(kt + 1) * 128],
                             cnt2[:, 0:1], start=True, stop=False)
            nc.tensor.matmul(sgq0_ps[:, kt:kt + 1],
                             objtab_lo[:, kt * 128:(kt + 1) * 128],
                             cnt2[0:32, 1:2], start=False, stop=True)
        sgq0 = pp.tile([128, 4], F32, name="sgq0", tag="sgq0")
        nc.vector.tensor_copy(sgq0[:], sgq0_ps[:])

        # ---- per-glimpse state ------------------------------------------
        # qrelu[g]: cols 0:512 = relu(QW) rows e~ 0..127;
        #           cols 512:1024 rows 0:32 = relu(QW) rows e~ 128..159
        qrelu = [pp.tile([128, 2 * MID], BF16, name=f"qrelu{g}", tag=f"qrelu{g}")
                 for g in range(GLIMPSES)]
        # A2 = A.T in [r, e~] layout, per 128-relation block (fp8)
        a2hi = pp.tile([128, NRB * 128], FP8, name="a2hi", tag="a2hi")
        a2lo = pp.tile([128, NRB * 32], FP8, name="a2lo", tag="a2lo")
        hrow = [pp.tile([1, MID], F32, name=f"hrow{g}", tag=f"hrow{g}")
                for g in range(GLIMPSES)]
        hT_all = [pp.tile([128, 4], F32, name=f"hTa{g}", tag=f"hTa{g}")
                  for g in range(GLIMPSES)]
        ah_sb = [pp.tile([128, 4], F32, name=f"ah{g}", tag=f"ah{g}") for g in range(GLIMPSES)]
        z1bq = pp.tile([1, MID], BF16, name="z1bq", tag="z1bq")
        ah_bf = pp.tile([128, 4], BF16, name="ah_bf", tag="ah_bf")
        vch1 = pp.tile([128, NRB * 512], FP8, name="vch1", tag="vch1")

        def emit_qw_acc(g, pools=None):
            # QW_g partial = obj_tab_pe @ WqT_g (accumulation, no bias yet)
            tiles = []
            for part, (sl_off, psz, ofree) in enumerate(((0, 128, 0),
                                                         (128, 32, MID))):
                pool = paux if pools is None else pools[part]
                tag = "aux" if pools is None else ["psA0", "pbl"][part]
                ps = pool.tile([128, MID], F32, name=f"qwps{g}_{sl_off}", tag=tag)
                for kb in range(4):
                    nc.tensor.matmul(
                        ps[0:psz, :],
                        objT[:, kb * NE + sl_off: kb * NE + sl_off + psz],
                        wqTg[g][:, kb * MID:(kb + 1) * MID],
                        start=(kb == 0), stop=False)
                tiles.append((ps, psz, ofree))
            return tiles

        def emit_qw_fin(g, bias_row, tiles):
            for ps, psz, ofree in tiles:
                nc.tensor.matmul(ps[0:psz, :], ones[0:1, 0:psz], bias_row,
                                 start=False, stop=True)
                nc.scalar.activation(qrelu[g][0:psz, ofree:ofree + MID],
                                     ps[0:psz, :], AT.Relu)

        def emit_A_mms(rc):
            # A[e~, r-chunk] via one-hot aggregation matmuls (fp8 DoubleRow)
            ps0 = pa0.tile([128, 512], F32, name=f"psA0_{rc}", tag="psA0")
            ps1 = paux.tile([128, 512], F32, name=f"psA1_{rc}", tag="aux")
            g8 = g8rc[rc]
            goff = 0
            b = 0
            for w, nb in enumerate(WIN_BLOCKS):
                out_ap = ps0[:] if w == 0 else ps1[0:32, :]
                wd = WIN_WIDTH[w]
                i = 0
                while i < nb:
                    if i + 2 <= nb:
                        nc.tensor.matmul(
                            out_ap,
                            wW[:, WCOL[w] + i * wd:
                               WCOL[w] + (i + 2) * wd].rearrange(
                                "p (k n) -> p k n", k=2),
                            g8[:, goff + (b + i) * 512:
                               goff + (b + i + 2) * 512].rearrange(
                                "p (k n) -> p k n", k=2),
                            start=(i == 0), stop=(i + 2 == nb),
                            perf_mode=mybir.MatmulPerfMode.DoubleRow)
                        i += 2
                    else:
                        nc.tensor.matmul(
                            out_ap,
                            wW[:, WCOL[w] + i * wd:WCOL[w] + (i + 1) * wd],
                            g8[:, goff + (b + i) * 512:goff + (b + i + 1) * 512],
                            start=(i == 0), stop=True)
                        i += 1
                b += nb
            return ps0, ps1

        def emit_A_post(rc, ps0, ps1):
            # copy out of PSUM, PE-transpose into a2hi/a2lo ([r, e~] fp8)
            atmp = wp.tile([128, 512], BF16, name=f"atmp{rc}", tag="atmp")
            nc.scalar.copy(atmp[:, 0:256], ps0[:, 0:256])
            nc.vector.tensor_copy(atmp[:, 256:512], ps0[:, 256:512])
            atlo = wp.tile([32, 512], BF16, name=f"atlo{rc}", tag="atlo")
            nc.vector.tensor_copy(atlo[:], ps1[0:32, :])
            ptr = paux.tile([128, 640], BF16, name=f"ptr{rc}", tag="aux")
            for i in range(4):
                nc.tensor.transpose(ptr[:, i * 128:(i + 1) * 128],
                                    atmp[:, i * 128:(i + 1) * 128], ident[:])
                nc.tensor.transpose(ptr[:, 512 + i * 32:512 + (i + 1) * 32],
                                    atlo[0:32, i * 128:(i + 1) * 128],
                                    ident[0:32, 0:32])
            nc.scalar.copy(a2hi[:, rc * 512:(rc + 1) * 512], ptr[:, 0:512])
            nc.vector.tensor_copy(a2lo[:, rc * 128:(rc + 1) * 128], ptr[:, 512:640])

        def emit_v(g, rb, dst):
            # v[r-block, m] = relu(oht.T-block @ abc_g), DoubleRow over the
            # 154-label contraction (Ki=77, ko-major)
            vps = pvv.tile([128, 512], F32, name=f"vps{g}{rb}", tag="vps")
            nc.tensor.matmul(
                vps[:],
                ohtdr[:, rb * 256:(rb + 1) * 256].rearrange(
                    "p (k n) -> p k n", k=2),
                abcdr[:, g * 1024:(g + 1) * 1024].rearrange(
                    "p (k n) -> p k n", k=2),
                start=True, stop=True,
                perf_mode=mybir.MatmulPerfMode.DoubleRow)
            if rb % 2 == 0:
                nc.scalar.activation(dst, vps[:], AT.Relu)
            else:
                nc.vector.tensor_scalar(dst, vps[:], 0.0, None, OP.max)

        def emit_B(g, pr, pbh, pbl, vchp):
            # B accumulation over relation blocks, DoubleRow over block pairs
            nc.tensor.matmul(
                pbh[:],
                a2hi[:, pr * 256:(pr + 1) * 256].rearrange("p (k n) -> p k n", k=2),
                vchp, start=(pr == 0), stop=(pr == NRB // 2 - 1),
                perf_mode=mybir.MatmulPerfMode.DoubleRow)
            nc.tensor.matmul(
                pbl[0:32, :],
                a2lo[:, pr * 64:(pr + 1) * 64].rearrange("p (k n) -> p k n", k=2),
                vchp, start=(pr == 0), stop=(pr == NRB // 2 - 1),
                perf_mode=mybir.MatmulPerfMode.DoubleRow)

        def emit_h_pre(g, pbh, pbl):
            # h[m] = sum_e qrelu[e,m] * B[e,m], then AllReduce the [1,512] row
            phi = wp.tile([128, 512], BF16, name=f"phi{g}", tag="phi")
            nc.vector.tensor_tensor(phi[:], pbh[:], qrelu[g][:, 0:MID], OP.mult)
            plo = wp.tile([32, 512], BF16, name=f"plo{g}", tag="plo")
            nc.vector.tensor_tensor(plo[:], pbl[0:32, :],
                                    qrelu[g][0:32, MID:2 * MID], OP.mult)
            hps = paux.tile([1, MID], F32, name=f"hps{g}", tag="aux")
            nc.tensor.matmul(hps[0:1, :], onesc[:, 0:1], phi[:],
                             start=True, stop=False)
            nc.tensor.matmul(hps[0:1, :], onesc[0:32, 0:1], plo[:],
                             start=False, stop=True)
            nc.vector.tensor_scalar(hrow[g][:], hps[0:1, :],
                                    1.0 / (CSCALE * VSCALE), None, OP.mult)
            h_in = dp.tile([1, MID], F32, name=f"h_in{g}", tag=f"h_in{g}")
            h_out = dp.tile([1, MID], F32, name=f"h_out{g}", tag=f"h_out{g}",
                            addr_space="Shared")
            nc.sync.dma_start(h_in[:], hrow[g][:])
            nc.gpsimd.collective_compute(
                "AllReduce", OP.add, replica_groups=rg,
                ins=[h_in[:].opt()], outs=[h_out[:].opt()])
            # layout-converting return DMA: [1, 512] row -> [128, 4] columns
            nc.sync.dma_start(
                hT_all[g][:],
                h_out[0:1, :].rearrange("a (c p) -> (a p) c", p=128))

        def emit_ah(g):
            # ahT = WaT_g.T-blocks @ hT + baT  (fp32, partition layout)
            aps = paux.tile([128, 4], F32, name=f"ahps{g}", tag="aux")
            for mt in range(4):
                for kb in range(4):
                    nc.tensor.matmul(
                        aps[:, mt:mt + 1],
                        waT[:, (g * 4 + kb) * MID + mt * 128:
                            (g * 4 + kb) * MID + (mt + 1) * 128],
                        hT_all[g][:, kb:kb + 1],
                        start=(kb == 0), stop=(kb == 3))
            nc.vector.tensor_tensor(ah_sb[g][:], aps[:], baT[:, g * 4:(g + 1) * 4], OP.add)

        # ================= schedule =====================================
        emit_qw_fin(0, bqrow[0:1, 0:MID], emit_qw_acc(0))
        pbh0 = pbhp.tile([128, 512], F32, name="pbh0", tag="pbh")
        pbl0 = pblp.tile([32, 512], F32, name="pbl0", tag="pbl")
        pbh1 = pbhp.tile([128, 512], F32, name="pbh1", tag="pbh")
        pbl1 = pblp.tile([32, 512], F32, name="pbl1", tag="pbl")
        psA = emit_A_mms(0)
        for rc in range(NRC):
            psA_next = emit_A_mms(rc + 1) if rc + 1 < NRC else None
            emit_A_post(rc, *psA)
            psA = psA_next
            for pi in range(2):
                pr = rc * 2 + pi
                vchp = wp.tile([128, 1024], FP8, name=f"vchp{pr}", tag="vchp")
                emit_v(0, 2 * pr, vchp[:, 0:512])
                emit_v(0, 2 * pr + 1, vchp[:, 512:1024])
                emit_B(0, pr, pbh0, pbl0,
                       vchp[:].rearrange("p (k n) -> p k n", k=2))
                # glimpse-1 v AND B also run in this phase (B1 is independent
                # of h0; only P1 = qrelu1 * B1 needs the AllReduce result)
                rb1 = rc * 2 + pi
                emit_v(1, rb1, vch1[:, rb1 * 512:(rb1 + 1) * 512])
                if pi == 1:
                    emit_B(1, rc, pbh1, pbl1,
                           vch1[:, rc * 1024:(rc + 1) * 1024].rearrange(
                               "p (k n) -> p k n", k=2))
        emit_h_pre(0, pbh0, pbl0)

        # overlap the h0 AllReduce: remaining glimpse-1 v/B and QW_1 accum
        qw1_tiles = emit_qw_acc(1, pools=(pa0, pblp))
        for rb in range(16, NRB):
            emit_v(1, rb, vch1[:, rb * 512:(rb + 1) * 512])
            if rb % 2 == 1:
                pr = rb // 2
                emit_B(1, pr, pbh1, pbl1,
                       vch1[:, pr * 1024:(pr + 1) * 1024].rearrange(
                           "p (k n) -> p k n", k=2))

        # z1 = h0 @ (Wa0.T @ Wq1.T) + (ba0 @ Wq1.T + bq1)  (host-folded)
        nc.scalar.copy(ah_bf[:], hT_all[0][:])
        zps = paux.tile([1, MID], F32, name="zps", tag="aux")
        for kb in range(4):
            nc.tensor.matmul(zps[:], ah_bf[:, kb:kb + 1],
                             wfold[:, kb * MID:(kb + 1) * MID],
                             start=(kb == 0), stop=(kb == 3))
        nc.vector.tensor_tensor(z1bq[:], zps[:], bqrow[0:1, MID:2 * MID], OP.add)

        emit_qw_fin(1, z1bq[0:1, :], qw1_tiles)
        emit_h_pre(1, pbh1, pbl1)
        emit_ah(0)              # overlaps the h1 AllReduce
        emit_ah(1)

        # sg = sgq0 + 2048*(ah0+ah1)    (partition layout [128,4])
        sgT = pp.tile([128, 4], F32, name="sgT", tag="sgT")
        nc.vector.tensor_tensor(sgT[:], ah_sb[0][:], ah_sb[1][:], OP.add)
        nc.vector.tensor_scalar(sgT[:], sgT[:], float(NOBJ), None, OP.mult)
        nc.vector.tensor_tensor(sgT[:], sgT[:], sgq0[:], OP.add)

        # fc1: o1T = relu(fc1T.T-blocks @ sgT + fc1bT)
        o1ps = paux.tile([128, 4], F32, name="o1ps", tag="aux")
        for jt in range(4):
            for kb in range(4):
                nc.tensor.matmul(
                    o1ps[:, jt:jt + 1],
                    fc1T[:, kb * MID + jt * 128: kb * MID + (jt + 1) * 128],
                    sgT[:, kb:kb + 1], start=(kb == 0), stop=(kb == 3))
        o1T = pp.tile([128, 4], BF16, name="o1T", tag="o1T")
        for jt in range(4):
            nc.scalar.activation(o1T[:, jt:jt + 1], o1ps[:, jt:jt + 1],
                                 AT.Relu, bias=fc1bT[:, jt:jt + 1])

        # fc2: out = relu(o1 @ fc2T + fc2b)   [1, 1024]
        out_sb = pp.tile([1, FINAL], F32, name="out_sb", tag="out_sb")
        for half in range(2):
            ops_ = paux.tile([1, 512], F32, name=f"ops{half}", tag="aux")
            for kb in range(4):
                nc.tensor.matmul(
                    ops_[:], o1T[:, kb:kb + 1],
                    fc2T[:, kb * FINAL + half * 512: kb * FINAL + half * 512 + 512],
                    start=(kb == 0), stop=(kb == 3))
            nc.vector.tensor_tensor(out_sb[0:1, half * 512:(half + 1) * 512],
                                    ops_[:], fc2b[0:1, half * 512:(half + 1) * 512],
                                    OP.add)
        nc.vector.tensor_scalar(out_sb[:], out_sb[:], 0.0, None, OP.max)
        nc.sync.dma_start(d["d_out"][:, :], out_sb[:])


def _pack_entities(cnt):
    """Pick 23 entities for window 1 such that their object count fits in
    [2048 - WIN_CAP[0], WIN_CAP[1]]; the other 128 go to window 0.
    Returns (win_ent lists, perm_e[151] -> e~: window0 -> 0..127,
    window1 -> 128..150)."""
    lo = int(cnt.sum()) - WIN_CAP[0]
    hi = WIN_CAP[1]
    order = np.argsort(cnt, kind="stable")          # ascending by count
    sums = np.convolve(cnt[order], np.ones(WIN_ENTS[1], dtype=np.int64),
                       "valid")                      # 23-entity window sums
    target = (lo + hi) / 2.0
    k = int(np.argmin(np.abs(sums - target)))
    s = int(sums[k])
    assert lo <= s <= hi, f"window packing failed: {s} not in [{lo},{hi}]"
    w1 = [int(e) for e in order[k:k + WIN_ENTS[1]]]
    w1set = set(w1)
    w0 = [e for e in range(151) if e not in w1set]
    perm_e = np.full(151, -1, np.int64)
    for c, e in enumerate(w0):
        perm_e[e] = c
    for c, e in enumerate(w1):
        perm_e[e] = 128 + c
    assert (perm_e >= 0).all()
    return [w0, w1], perm_e


def _prep_inputs(entities, relations, graph, obj_tab, head_tab, tail_tab, pred_tab,
                 lin_v_v, lin_v_g, lin_v_b, lin_q_v, lin_q_g, lin_q_b,
                 lin_a_v, lin_a_g, lin_a_b, fc1_w, fc1_b, fc2_w, fc2_b):
    ent = np.asarray(entities).astype(np.int64)
    rel = np.asarray(relations).astype(np.int64)
    graph = np.asarray(graph, dtype=np.float32)

    abc = np.zeros((GLIMPSES, 153, MID), np.float32)
    wqT = np.zeros((GLIMPSES, EMBED, MID), np.float32)
    waT = np.zeros((GLIMPSES, MID, MID), np.float32)
    baT = np.zeros((GLIMPSES, 128, 4), np.float32)
    bq = np.zeros((GLIMPSES, MID), np.float32)
    for g in range(GLIMPSES):
        Wv = _wn(np.asarray(lin_v_v[g], np.float32), float(lin_v_g[g]))
        abc[g, 0:51] = head_tab[:51] @ Wv[:, 0:EMBED].T + np.asarray(lin_v_b[g], np.float32)
        abc[g, 51:102] = tail_tab[:51] @ Wv[:, EMBED:2 * EMBED].T
        abc[g, 102:153] = pred_tab[:51] @ Wv[:, 2 * EMBED:3 * EMBED].T
        Wq = _wn(np.asarray(lin_q_v[g], np.float32), float(lin_q_g[g]))
        wqT[g] = Wq.T
        bq[g] = np.asarray(lin_q_b[g], np.float32)
        Wa = _wn(np.asarray(lin_a_v[g], np.float32), float(lin_a_g[g]))
        waT[g] = Wa.T
        baT[g] = np.asarray(lin_a_b[g], np.float32).reshape(4, 128).T

    # relation one-hots (unchanged relation order, sharded by core)
    oht = np.zeros((NCORES, 153, RCH), NP_BF16)
    ar = np.arange(RCH)
    for c in range(NCORES):
        rc_ = rel[c * RCH:(c + 1) * RCH]
        m = np.zeros((153, RCH), np.float32)
        m[rc_[:, 0], ar] = 1.0
        m[rc_[:, 1] + 51, ar] = 1.0
        m[rc_[:, 2] + 102, ar] = 1.0
        oht[c] = m.astype(NP_BF16)

    # ---- entity windows, object sort/pad, W matrix ----------------------
    cnt = np.bincount(ent, minlength=151).astype(np.int64)
    win_ent, perm_e = _pack_entities(cnt)
    recip = (CSCALE / (graph.sum(axis=1, dtype=np.float32) + 1e-9)).astype(np.float32)

    obj_order = np.full(NBLK * 128, -1, np.int64)
    slot_ecol = np.full(NBLK * 128, -1, np.int64)    # window-local e column
    slot_rec = np.zeros(NBLK * 128, np.float32)
    by_ent = [np.nonzero(ent == e)[0] for e in range(151)]
    base = 0
    for w in range(2):
        pos = 0
        for c, e in enumerate(win_ent[w]):
            objs = by_ent[e]
            sl = slice(base + pos, base + pos + len(objs))
            obj_order[sl] = objs
            slot_ecol[sl] = c
            slot_rec[sl] = recip[objs]
            pos += len(objs)
        base += WIN_CAP[w]
    # W device layout [128, NWCOL]: per-window pieces of width WIN_WIDTH[w]
    W3 = np.zeros((128, NWCOL), np.float32)
    blk = 0
    for w in range(2):
        wd = WIN_WIDTH[w]
        for pb in range(WIN_BLOCKS[w]):
            rows = np.arange(blk * 128, (blk + 1) * 128)
            valid_r = slot_ecol[rows] >= 0
            pr = np.nonzero(valid_r)[0]
            W3[pr, WCOL[w] + pb * wd + slot_ecol[rows[pr]]] = slot_rec[rows[pr]]
            blk += 1

    graph_s = np.zeros((NBLK * 128, NREL), NP_FP8)
    valid = obj_order >= 0
    graph_s[valid] = graph[obj_order[valid]].astype(NP_FP8)

    # permuted entity tables (padded to NE)
    obj_tab_pe = np.zeros((NE, EMBED), np.float32)
    cnt_pe = np.zeros(NE, np.float32)
    obj_tab_pe[perm_e] = np.asarray(obj_tab, np.float32)
    cnt_pe[perm_e] = cnt
    cnt2 = np.zeros((128, 2), np.float32)
    cnt2[:, 0] = cnt_pe[0:128]
    cnt2[0:32, 1] = cnt_pe[128:NE]
    objT = np.ascontiguousarray(obj_tab_pe.T).reshape(4, 128, NE)

    # ---- coalesced packs -------------------------------------------------
    wpack = np.zeros((128, NW), np.float32)
    wpack[:, OBJT:OBJT + 4 * NE] = np.concatenate([objT[kb] for kb in range(4)], axis=1)
    wpack[:, WQT0:WQT0 + 4 * MID] = np.concatenate(
        [wqT[0, kb * 128:(kb + 1) * 128, :] for kb in range(4)], axis=1)
    wpack[:, WQT1:WQT1 + 4 * MID] = np.concatenate(
        [wqT[1, kb * 128:(kb + 1) * 128, :] for kb in range(4)], axis=1)
    wpack[:, OTAB_HI:OTAB_HI + EMBED] = obj_tab_pe[0:128]
    wpack[0:32, OTAB_LO:OTAB_LO + EMBED] = obj_tab_pe[128:NE]
    wpack[:, IDENT:IDENT + 128] = np.eye(128, dtype=np.float32)
    fc2T = np.ascontiguousarray(fc2_w.astype(np.float32).T)
    wpack[:, FC2TC:FC2TC + 4 * FINAL] = np.concatenate(
        [fc2T[kb * 128:(kb + 1) * 128, :] for kb in range(4)], axis=1)
    wfold = np.ascontiguousarray((waT[0] @ wqT[1]))     # Wa0.T @ Wq1.T
    wpack[:, WFOLD:WFOLD + 4 * MID] = np.concatenate(
        [wfold[kb * 128:(kb + 1) * 128, :] for kb in range(4)], axis=1)
    wpack[:, CNT2:CNT2 + 2] = cnt2
    wpack[:, ONESC:ONESC + 1] = 1.0

    fpack = np.zeros((128, NF), np.float32)
    fpack[:, WAT:WAT + GLIMPSES * 4 * MID] = np.concatenate(
        [waT[g, kb * 128:(kb + 1) * 128, :] for g in range(GLIMPSES) for kb in range(4)],
        axis=1)
    fc1T = np.ascontiguousarray(fc1_w.astype(np.float32).T)
    fpack[:, FC1T:FC1T + 4 * MID] = np.concatenate(
        [fc1T[kb * 128:(kb + 1) * 128, :] for kb in range(4)], axis=1)
    fpack[:, BAT:BAT + GLIMPSES * 4] = np.concatenate(
        [baT[g] for g in range(GLIMPSES)], axis=1)
    fpack[:, FC1BT:FC1BT + 4] = np.asarray(fc1_b, np.float32).reshape(4, 128).T
    fpack[:, ONESF:ONESF + 1] = 1.0

    rowpack = np.zeros((1, NR), np.float32)
    rowpack[0, 0:MID] = bq[0]
    # glimpse-1 bias row folded with ba0 @ Wq1.T (z1 shortcut)
    rowpack[0, MID:2 * MID] = (np.asarray(lin_a_b[0], np.float32) @ wqT[1]
                               + bq[1])
    rowpack[0, RONES:RONES + 128] = 1.0

    # abc tables, fp8 DoubleRow layout [77, g*1024 + ko*512 + m]
    abc_pad = np.zeros((154, GLIMPSES, MID), np.float32)
    abc_pad[0:153] = (abc * VSCALE).transpose(1, 0, 2)
    abcdr = abc_pad.reshape(2, 77, GLIMPSES, MID).transpose(1, 2, 0, 3)
    abcdr = np.ascontiguousarray(abcdr.reshape(77, GLIMPSES * 1024))

    base_map = {
        "W": W3.astype(NP_FP8),
        "wpack": wpack.astype(NP_BF16),
        "fpack": fpack,
        "rowpack": rowpack.astype(NP_BF16),
        "fc2b": np.asarray(fc2_b, np.float32).reshape(1, FINAL),
        "abcdr": abcdr.astype(NP_FP8),
    }
    in_maps = []
    for c in range(NCORES):
        m = dict(base_map)
        gs = graph_s[:, c * RCH:(c + 1) * RCH]            # [NBLK*128, RCH]
        gs = gs.reshape(NBLK, 128, NRC, 512).transpose(2, 1, 0, 3)
        m["g"] = np.ascontiguousarray(gs.reshape(NRC, 128, NBLK * 512))
        # oht one-hots, fp8 DoubleRow layout [77, rb*256 + ko*128 + j]
        oht_pad = np.zeros((154, RCH), np.float32)
        oht_pad[0:153] = oht[c].astype(np.float32)
        ohtdr = oht_pad.reshape(2, 77, NRB, 128).transpose(1, 2, 0, 3)
        m["ohtdr"] = np.ascontiguousarray(
            ohtdr.reshape(77, 2 * RCH)).astype(NP_FP8)
        in_maps.append(m)
    return in_maps


def kernel(**inputs):
    if "nc" not in _CACHE:
        _CACHE["nc"], _CACHE["in_names"] = _build()
    nc = _CACHE["nc"]
    in_maps = _prep_inputs(**inputs)
    res = bass_utils.run_bass_kernel_spmd(nc, in_maps, core_ids=list(range(NCORES)))
    return np.asarray(res.results[0]["out"], np.float32)



# revision 26
# speedup vs baseline: 1.5406x; 1.5406x over previous
import sys

for _p in ("/opt/trn_rl_repo",):
    if _p not in sys.path:
        sys.path.insert(0, _p)

import numpy as np
import ml_dtypes

import concourse.bass as bass
import concourse.bacc as bacc
import concourse.tile as tile
import concourse.mybir as mybir
from concourse import bass_utils

F32 = mybir.dt.float32
BF16 = mybir.dt.bfloat16
FP8 = mybir.dt.float8e4

NP_BF16 = ml_dtypes.bfloat16
NP_FP8 = ml_dtypes.float8_e4m3

EMBED = 512
MID = 512
FINAL = 1024
GLIMPSES = 2
NOBJ = 2048
NREL = 32768
NCORES = 8
RCH = NREL // NCORES          # 4096 relations per core
NRC = RCH // 512              # 8 relation chunks of 512 per core
NRB = RCH // 128              # 32 relation blocks of 128 per core
NBLK = NOBJ // 128            # 16 object blocks, no padding
NPAIR = NBLK // 2             # 8 object-block pairs (fp8 DoubleRow)
NE = 160                      # padded entity count (151 -> 160)
CSCALE = 65536.0              # fp8 range scaling for recip in wT
VSCALE = 1024.0               # fp8 range scaling for abc tables
CV = CSCALE * VSCALE
F2SCALE = 1.0                 # fc2 runs in bf16; no range scaling

# qpack (bf16) column offsets
QR0H = 0                      # qrelu0/CV rows 0:128
QR0L = QR0H + MID             # qrelu0/CV rows 128:160 (on partitions 0:32)
Z1SH = QR0L + MID             # z1 static/CV rows 0:128
Z1SL = Z1SH + MID             # z1 static/CV rows 128:160
IDENT = Z1SL + MID            # 128 identity
ONESC = IDENT + 128           # 1 column of ones
NQ = ONESC + 1

# lpack (bf16) column offsets: wfold/CV k-blocks, then Fpack (g, kb, jt)
WFOLD = 0                     # 4 * MID
FPK = WFOLD + 4 * MID         # 2 * 4 * 4 * 128 = 4096
NL = FPK + GLIMPSES * 16 * 128

# rowp (bf16) [1, x]: ones 0:128, fc2b*F2SCALE 128:1152
RONES = 0
RB32 = 128
NR = RB32 + FINAL

_CACHE = {}


def _wn(v, g):
    return (v * (g / np.linalg.norm(v.astype(np.float64)))).astype(np.float32)


def _build():
    """Builds the Bass program once. Returns (nc, input tensor names)."""
    nc = bacc.Bacc(
        "TRN2",
        target_bir_lowering=False,
        debug=False,
        enable_asserts=False,
        num_devices=NCORES,
    )

    # ---- DRAM I/O -------------------------------------------------------
    d_g = nc.dram_tensor("g", [NRC, 128, NBLK * 512], FP8, kind="ExternalInput")
    d_wT = nc.dram_tensor("wT", [128, NPAIR * 2 * NE], FP8, kind="ExternalInput")
    d_ohtdr = nc.dram_tensor("ohtdr", [77, 2 * RCH], FP8, kind="ExternalInput")
    d_abcdr = nc.dram_tensor("abcdr", [77, GLIMPSES * 1024], FP8,
                             kind="ExternalInput")
    d_qpack = nc.dram_tensor("qpack", [128, NQ], BF16, kind="ExternalInput")
    d_lpack = nc.dram_tensor("lpack", [128, NL], BF16, kind="ExternalInput")
    d_fc2dr = nc.dram_tensor("fc2dr", [128, 4 * FINAL], BF16,
                             kind="ExternalInput")
    d_bpack = nc.dram_tensor("bpack", [128, 5], F32, kind="ExternalInput")
    d_rowp = nc.dram_tensor("rowp", [1, NR], BF16, kind="ExternalInput")
    d_out = nc.dram_tensor("out", [1, FINAL], F32, kind="ExternalOutput")

    with tile.TileContext(nc) as tc:
        _emit(nc, tc, locals())

    nc.compile()
    in_names = ["g", "wT", "ohtdr", "abcdr", "qpack", "lpack", "fc2dr",
                "bpack", "rowp"]
    return nc, in_names


def _emit(nc, tc, d):
    AT = mybir.ActivationFunctionType
    OP = mybir.AluOpType
    rg = [list(range(NCORES))]

    with (
        tc.tile_pool(name="persist", bufs=1) as pp,
        tc.tile_pool(name="work", bufs=6) as wp,
        tc.tile_pool(name="gpool", bufs=5) as gp,
        tc.tile_pool(name="pvv", bufs=2, space="PSUM") as pvv,
        tc.tile_pool(name="pat", bufs=2, space="PSUM") as pat,
        tc.tile_pool(name="pbh", bufs=2, space="PSUM") as pbhp,
        tc.tile_pool(name="plo", bufs=2, space="PSUM") as plop,
        tc.tile_pool(name="dram", bufs=1, space="DRAM") as dp,
    ):
        # ---- input DMAs: one queue (SP), strict priority order ----------
        wT = pp.tile([128, NPAIR * 2 * NE], FP8, name="wT", tag="wT")
        nc.sync.dma_start(wT[:], d["d_wT"][:, :])
        ohtdr = pp.tile([77, 2 * RCH], FP8, name="ohtdr", tag="ohtdr")
        abcdr = pp.tile([77, GLIMPSES * 1024], FP8, name="abcdr", tag="abcdr")
        qpk = pp.tile([128, NQ], BF16, name="qpk", tag="qpk")
        g8rc = []
        for rc in range(NRC):
            t = gp.tile([128, NBLK * 512], FP8, name=f"g8rc{rc}", tag="g8")
            nc.sync.dma_start(t[:], d["d_g"][rc])
            g8rc.append(t)
            if rc == 0:
                nc.sync.dma_start(ohtdr[:], d["d_ohtdr"][:, :])
                nc.sync.dma_start(abcdr[:], d["d_abcdr"][:, :])
            elif rc == 1:
                nc.sync.dma_start(qpk[:], d["d_qpack"][:, :])

        lpk = pp.tile([128, NL], BF16, name="lpk", tag="lpk")
        nc.sync.dma_start(lpk[:], d["d_lpack"][:, :])
        fc2dr = pp.tile([128, 4 * FINAL], BF16, name="fc2dr", tag="fc2dr")
        nc.sync.dma_start(fc2dr[:], d["d_fc2dr"][:, :])
        bpack = pp.tile([128, 5], F32, name="bpack", tag="bpack")
        nc.sync.dma_start(bpack[:], d["d_bpack"][:, :])
        rowp = pp.tile([1, NR], BF16, name="rowp", tag="rowp")
        nc.sync.dma_start(rowp[:], d["d_rowp"][:, :])

        # ---- views into the packs ---------------------------------------
        qr0hi = qpk[:, QR0H:QR0H + MID]
        qr0lo = qpk[0:32, QR0L:QR0L + MID]
        z1shi = qpk[:, Z1SH:Z1SH + MID]
        z1slo = qpk[0:32, Z1SL:Z1SL + MID]
        ident = qpk[:, IDENT:IDENT + 128]
        onesc = qpk[:, ONESC:ONESC + 1]
        wfold = lpk[:, WFOLD:WFOLD + 4 * MID]
        fpk = lpk[:, FPK:FPK + GLIMPSES * 16 * 128]
        ones_row = rowp[0:1, RONES:RONES + 128]
        fc2b32 = rowp[0:1, RB32:RB32 + FINAL]
        baseT = bpack[:, 0:4]
        onesf = bpack[:, 4:5]

        # ---- persistent state -------------------------------------------
        a2hi = pp.tile([128, NRB * 128], FP8, name="a2hi", tag="a2hi")
        a2lo = pp.tile([128, NRB * 32], FP8, name="a2lo", tag="a2lo")
        hbf = [pp.tile([128, 4], BF16, name=f"hbf{g}", tag=f"hbf{g}")
               for g in range(GLIMPSES)]
        qr1hi = pp.tile([128, MID], BF16, name="qr1hi", tag="qr1hi")
        qr1lo = pp.tile([32, MID], BF16, name="qr1lo", tag="qr1lo")
        zrow = pp.tile([1, MID], BF16, name="zrow", tag="zrow")
        o1f = pp.tile([128, 4], F32, name="o1f", tag="o1f")
        o1T8 = pp.tile([128, 4], BF16, name="o1T8", tag="o1T8")
        out_sb = pp.tile([1, FINAL], F32, name="out_sb", tag="out_sb")

        pbh = [pbhp.tile([128, 512], F32, name=f"pbh{g}", tag="pbh")
               for g in range(GLIMPSES)]
        pbl = [plop.tile([32, 512], F32, name=f"pbl{g}", tag="plo")
               for g in range(GLIMPSES)]

        # copy engines (GPSIMD cannot access PSUM, so only Act/DVE here)
        def relu_copy(eng_i, dst, src):
            if eng_i == 0:
                nc.scalar.activation(dst, src, AT.Relu)
            else:
                nc.vector.tensor_scalar(dst, src, 0.0, None, OP.max)

        def plain_copy(eng_i, dst, src):
            if eng_i == 0:
                nc.scalar.copy(dst, src)
            else:
                nc.vector.tensor_copy(dst, src)

        # ================= streaming phase ===============================
        # The stream carries only glimpse-0 work (A^T aggregation, v0, B0);
        # glimpse-1 v/B runs later, hidden under the first AllGather.
        # B matmuls for chunk rc are emitted during chunk rc+1 (PE executes
        # in-order; the one-chunk delay guarantees the v relu-copies feeding
        # them are long done, so PE never stalls on the vector engines).
        def emit_v(g, rb, dst, eng_i):
            vps = pvv.tile([128, 512], F32, name=f"vps{g}{rb}", tag="vps")
            nc.tensor.matmul(
                vps[:],
                ohtdr[:, rb * 256:(rb + 1) * 256].rearrange(
                    "p (k n) -> p k n", k=2),
                abcdr[:, g * 1024:(g + 1) * 1024].rearrange(
                    "p (k n) -> p k n", k=2),
                start=True, stop=True,
                perf_mode=mybir.MatmulPerfMode.DoubleRow)
            relu_copy(eng_i, dst, vps[:])

        def emit_B(g, pr, vpair_t):
            vpair = vpair_t[:].rearrange("p (k n) -> p k n", k=2)
            nc.tensor.matmul(
                pbh[g][:],
                a2hi[:, pr * 256:(pr + 1) * 256].rearrange(
                    "p (k n) -> p k n", k=2),
                vpair, start=(pr == 0), stop=(pr == NRB // 2 - 1),
                perf_mode=mybir.MatmulPerfMode.DoubleRow)
            nc.tensor.matmul(
                pbl[g][:],
                a2lo[:, pr * 64:(pr + 1) * 64].rearrange(
                    "p (k n) -> p k n", k=2),
                vpair, start=(pr == 0), stop=(pr == NRB // 2 - 1),
                perf_mode=mybir.MatmulPerfMode.DoubleRow)

        vchp = {}
        pending_B = []
        for rc in range(NRC):
            g9 = g8rc[rc]
            for j0 in range(4):
                rb = rc * 4 + j0
                if j0 in (0, 2) and rc > 0:
                    emit_B(0, *pending_B.pop(0))
                psAT = pat.tile([128, NE], F32, name=f"psAT{rb}", tag="at")
                for pb in range(NPAIR):
                    nc.tensor.matmul(
                        psAT[:],
                        g9[:, j0 * 2048 + pb * 256:
                           j0 * 2048 + (pb + 1) * 256].rearrange(
                            "p (k n) -> p k n", k=2),
                        wT[:, pb * 2 * NE:(pb + 1) * 2 * NE].rearrange(
                            "p (k n) -> p k n", k=2),
                        start=(pb == 0), stop=(pb == NPAIR - 1),
                        perf_mode=mybir.MatmulPerfMode.DoubleRow)
                plain_copy(0, a2hi[:, rb * 128:(rb + 1) * 128],
                           psAT[:, 0:128])
                plain_copy(0, a2lo[:, rb * 32:(rb + 1) * 32],
                           psAT[:, 128:160])
                pr = rb // 2
                half = rb % 2
                if half == 0:
                    vchp[pr] = wp.tile([128, 1024], FP8, name=f"vch0_{pr}",
                                       tag="vch0")
                emit_v(0, rb, vchp[pr][:, half * 512:(half + 1) * 512], 1)
                if half == 1:
                    pending_B.append((pr, vchp[pr]))
        while pending_B:
            emit_B(0, *pending_B.pop(0))

        # ---- h partial for glimpse g (scales folded on host) -------------
        def emit_h(g, qhi, qlo):
            plo_t = wp.tile([32, 512], BF16, name=f"plo{g}", tag="plo")
            nc.vector.tensor_tensor(plo_t[:], pbl[g][:], qlo, OP.mult)
            phi = wp.tile([128, 512], BF16, name=f"phi{g}", tag="phi")
            nc.vector.tensor_tensor(phi[:], pbh[g][:], qhi, OP.mult)
            hps = pat.tile([1, MID], F32, name=f"hps{g}", tag="at")
            nc.tensor.matmul(hps[0:1, :], onesc[0:32, 0:1], plo_t[:],
                             start=True, stop=False)
            nc.tensor.matmul(hps[0:1, :], onesc[:, 0:1], phi[:],
                             start=False, stop=True)
            hsb = wp.tile([1, MID], F32, name=f"hsb{g}", tag="hsb")
            nc.scalar.copy(hsb[:], hps[0:1, :])
            h_in = dp.tile([1, MID], F32, name=f"h_in{g}", tag=f"h_in{g}")
            h_out = dp.tile([NCORES, MID], F32, name=f"h_out{g}",
                            tag=f"h_out{g}", addr_space="Shared")
            nc.sync.dma_start(h_in[:], hsb[:])
            nc.gpsimd.collective_compute(
                "AllGather", OP.bypass, replica_groups=rg,
                ins=[h_in[:].opt()], outs=[h_out[:].opt()])
            return h_out

        # Gathered [8, 512] -> natural-layout DMA, then 4 matmul-reduces
        # against a ones column -> hT in [128, 4] partition layout.
        def emit_h_sum(g, h_out):
            hgn = wp.tile([NCORES, MID], F32, name=f"hgn{g}", tag="hgn")
            nc.sync.dma_start(hgn[:], h_out[:, :])
            htp = pat.tile([128, 4], F32, name=f"htp{g}", tag="at")
            for c in range(4):
                nc.tensor.matmul(htp[:, c:c + 1],
                                 hgn[:, c * 128:(c + 1) * 128],
                                 onesf[0:NCORES, :], start=True, stop=True)
            nc.scalar.copy(hbf[g][:], htp[:])

        h_out0 = emit_h(0, qr0hi, qr0lo)

        # glimpse-1 v/B, hidden under the first AllGather's latency.
        # Copies go to Act/DVE only: the Pool queue head is blocked by the
        # collective_compute instruction until the AllGather completes.
        for pr in range(NRB // 2):
            vt = wp.tile([128, 1024], FP8, name=f"vch1_{pr}", tag="vch1")
            emit_v(1, 2 * pr, vt[:, 0:512], 0)
            emit_v(1, 2 * pr + 1, vt[:, 512:1024], 1)
            emit_B(1, pr, vt)

        emit_h_sum(0, h_out0)

        # ---- z1 row = h0 @ (wfold/CV) ------------------------------------
        zps = pat.tile([1, MID], F32, name="zps", tag="at")
        for kb in range(4):
            nc.tensor.matmul(zps[:], hbf[0][:, kb:kb + 1],
                             wfold[:, kb * MID:(kb + 1) * MID],
                             start=(kb == 0), stop=(kb == 3))
        nc.vector.tensor_copy(zrow[:], zps[:])

        # qrelu1 = relu(z1static + 1 (x) zrow), hi/lo via identity matmuls
        zbh = pvv.tile([128, MID], F32, name="zbh", tag="vps")
        nc.tensor.matmul(zbh[:], ident[:], z1shi, start=True, stop=False)
        nc.tensor.matmul(zbh[:], ones_row[0:1, 0:128], zrow[:],
                         start=False, stop=True)
        nc.scalar.activation(qr1hi[:], zbh[:], AT.Relu)
        zbl = pvv.tile([32, MID], F32, name="zbl", tag="vps")
        nc.tensor.matmul(zbl[:], ident[0:32, 0:32], z1slo,
                         start=True, stop=False)
        nc.tensor.matmul(zbl[:], ones_row[0:1, 0:32], zrow[:],
                         start=False, stop=True)
        nc.vector.tensor_scalar(qr1lo[:], zbl[:], 0.0, None, OP.max)

        h_out1 = emit_h(1, qr1hi[:], qr1lo[:])

        # ---- tail: o1 = relu(base + h0@F0 + h1@F1) ------------------------
        # h0@F0 runs under the second AllGather.
        o1ps = pvv.tile([128, 4], F32, name="o1ps", tag="vps")
        for jt in range(4):
            for kb in range(4):
                nc.tensor.matmul(
                    o1ps[:, jt:jt + 1],
                    fpk[:, (0 * 4 + kb) * 512 + jt * 128:
                        (0 * 4 + kb) * 512 + (jt + 1) * 128],
                    hbf[0][:, kb:kb + 1],
                    start=(kb == 0), stop=False)

        emit_h_sum(1, h_out1)
        for jt in range(4):
            for kb in range(4):
                nc.tensor.matmul(
                    o1ps[:, jt:jt + 1],
                    fpk[:, (1 * 4 + kb) * 512 + jt * 128:
                        (1 * 4 + kb) * 512 + (jt + 1) * 128],
                    hbf[1][:, kb:kb + 1],
                    start=False, stop=(kb == 3))
        nc.vector.tensor_tensor(o1f[:], o1ps[:], baseT, OP.add)
        nc.vector.tensor_scalar(o1T8[:], o1f[:], 0.0, None, OP.max)

        # fc2 in two 512-wide halves (one PSUM bank each), fp8 DoubleRow,
        # fc2b*F2SCALE folded in via a rank-1 matmul; final relu+unscale is
        # a single op per half (Act for half 0, DVE for half 1).
        for h in range(2):
            ops = pat.tile([1, 512], F32, name=f"ops{h}", tag="at")
            for kb in range(4):
                nc.tensor.matmul(
                    ops[:], o1T8[:, kb:kb + 1],
                    fc2dr[:, kb * FINAL + h * 512:kb * FINAL + h * 512 + 512],
                    start=(kb == 0), stop=False)
            nc.tensor.matmul(ops[:], ones_row[0:1, 0:1],
                             fc2b32[0:1, h * 512:(h + 1) * 512],
                             start=False, stop=True)
            osl = out_sb[0:1, h * 512:(h + 1) * 512]
            if h == 0:
                nc.scalar.activation(osl, ops[:], AT.Relu, scale=1.0 / F2SCALE)
            else:
                nc.vector.tensor_scalar(osl, ops[:], 1.0 / F2SCALE, 0.0,
                                        OP.mult, OP.max)
        nc.sync.dma_start(d["d_out"][:, :], out_sb[:])


def _prep_inputs(entities, relations, graph, obj_tab, head_tab, tail_tab, pred_tab,
                 lin_v_v, lin_v_g, lin_v_b, lin_q_v, lin_q_g, lin_q_b,
                 lin_a_v, lin_a_g, lin_a_b, fc1_w, fc1_b, fc2_w, fc2_b):
    ent = np.asarray(entities).astype(np.int64)
    rel = np.asarray(relations).astype(np.int64)
    graph = np.asarray(graph, dtype=np.float32)

    abc = np.zeros((GLIMPSES, 153, MID), np.float32)
    wqT = np.zeros((GLIMPSES, EMBED, MID), np.float32)
    waT = np.zeros((GLIMPSES, MID, MID), np.float32)
    ba = np.zeros((GLIMPSES, MID), np.float32)
    bq = np.zeros((GLIMPSES, MID), np.float32)
    for g in range(GLIMPSES):
        Wv = _wn(np.asarray(lin_v_v[g], np.float32), float(lin_v_g[g]))
        abc[g, 0:51] = head_tab[:51] @ Wv[:, 0:EMBED].T + np.asarray(lin_v_b[g], np.float32)
        abc[g, 51:102] = tail_tab[:51] @ Wv[:, EMBED:2 * EMBED].T
        abc[g, 102:153] = pred_tab[:51] @ Wv[:, 2 * EMBED:3 * EMBED].T
        Wq = _wn(np.asarray(lin_q_v[g], np.float32), float(lin_q_g[g]))
        wqT[g] = Wq.T
        bq[g] = np.asarray(lin_q_b[g], np.float32)
        Wa = _wn(np.asarray(lin_a_v[g], np.float32), float(lin_a_g[g]))
        waT[g] = Wa.T
        ba[g] = np.asarray(lin_a_b[g], np.float32)

    obj_tab = np.asarray(obj_tab, np.float32)
    fc1_b = np.asarray(fc1_b, np.float32)
    fc2_b = np.asarray(fc2_b, np.float32)
    fc1T = np.ascontiguousarray(fc1_w.astype(np.float32).T)   # [512, 512]
    fc2T = np.ascontiguousarray(fc2_w.astype(np.float32).T)   # [512, 1024]

    # relation one-hots (per core), fp8 DoubleRow layout [77, rb*256 + ko*128 + j]
    oht_all = []
    ar = np.arange(RCH)
    for c in range(NCORES):
        rc_ = rel[c * RCH:(c + 1) * RCH]
        m = np.zeros((154, RCH), np.float32)
        m[rc_[:, 0], ar] = 1.0
        m[rc_[:, 1] + 51, ar] = 1.0
        m[rc_[:, 2] + 102, ar] = 1.0
        ohtdr = m.reshape(2, 77, NRB, 128).transpose(1, 2, 0, 3)
        oht_all.append(np.ascontiguousarray(
            ohtdr.reshape(77, 2 * RCH)).astype(NP_FP8))

    # abc tables, fp8 DoubleRow layout [77, g*1024 + ko*512 + m]
    abc_pad = np.zeros((154, GLIMPSES, MID), np.float32)
    abc_pad[0:153] = (abc * VSCALE).transpose(1, 0, 2)
    abcdr = abc_pad.reshape(2, 77, GLIMPSES, MID).transpose(1, 2, 0, 3)
    abcdr = np.ascontiguousarray(abcdr.reshape(77, GLIMPSES * 1024)).astype(NP_FP8)

    # wT: scatter matrix [obj, entity] with recip scaling, DoubleRow layout
    recip = (CSCALE / (graph.sum(axis=1, dtype=np.float32) + 1e-9)).astype(np.float32)
    wTf = np.zeros((NOBJ, NE), np.float32)
    wTf[np.arange(NOBJ), ent] = recip
    wTdr = wTf.reshape(NPAIR, 2, 128, NE).transpose(2, 0, 1, 3)
    wTdr = np.ascontiguousarray(wTdr.reshape(128, NPAIR * 2 * NE)).astype(NP_FP8)

    # per-entity tables, scaled by 1/CV so device h comes out unscaled
    qr0 = np.zeros((NE, MID), np.float32)
    qr0[0:151] = np.maximum(obj_tab @ wqT[0] + bq[0], 0.0) * (1.0 / CV)
    z1s = np.zeros((NE, MID), np.float32)
    z1s[0:151] = (obj_tab @ wqT[1] + (ba[0] @ wqT[1] + bq[1])) * (1.0 / CV)

    qpack = np.zeros((128, NQ), np.float32)
    qpack[:, QR0H:QR0H + MID] = qr0[0:128]
    qpack[0:32, QR0L:QR0L + MID] = qr0[128:160]
    qpack[:, Z1SH:Z1SH + MID] = z1s[0:128]
    qpack[0:32, Z1SL:Z1SL + MID] = z1s[128:160]
    qpack[:, IDENT:IDENT + 128] = np.eye(128, dtype=np.float32)
    qpack[:, ONESC:ONESC + 1] = 1.0

    # tail folding: sgq0 = sum over objects of q0 rows
    cnt = np.bincount(ent, minlength=151).astype(np.float32)
    sgq0 = cnt @ obj_tab
    base = (sgq0 + NOBJ * (ba[0] + ba[1])) @ fc1T + fc1_b    # [512]
    bpack = np.zeros((128, 5), np.float32)
    bpack[:, 0:4] = base.reshape(4, 128).T
    bpack[:, 4] = 1.0
    F = [NOBJ * (waT[g] @ fc1T) for g in range(GLIMPSES)]    # [512, 512] each

    lpack = np.zeros((128, NL), np.float32)
    wfold = (waT[0] @ wqT[1]) * (1.0 / CV)
    lpack[:, WFOLD:WFOLD + 4 * MID] = np.concatenate(
        [wfold[kb * 128:(kb + 1) * 128, :] for kb in range(4)], axis=1)
    fblocks = []
    for g in range(GLIMPSES):
        for kb in range(4):
            for jt in range(4):
                fblocks.append(F[g][kb * 128:(kb + 1) * 128,
                                    jt * 128:(jt + 1) * 128])
    lpack[:, FPK:FPK + GLIMPSES * 16 * 128] = np.concatenate(fblocks, axis=1)

    # fc2 weights, fp8 k-block layout [128, kb*1024 + n]
    fc2s = fc2T.reshape(4, 128, FINAL).transpose(1, 0, 2)
    fc2dr = np.ascontiguousarray(
        fc2s.reshape(128, 4 * FINAL)).astype(NP_BF16)

    rowp = np.zeros((1, NR), np.float32)
    rowp[0, RONES:RONES + 128] = 1.0
    rowp[0, RB32:RB32 + FINAL] = fc2_b

    base_map = {
        "wT": wTdr,
        "abcdr": abcdr,
        "qpack": qpack.astype(NP_BF16),
        "lpack": lpack.astype(NP_BF16),
        "fc2dr": fc2dr,
        "bpack": bpack,
        "rowp": rowp.astype(NP_BF16),
    }
    in_maps = []
    for c in range(NCORES):
        m = dict(base_map)
        gs = graph[:, c * RCH:(c + 1) * RCH].astype(NP_FP8)   # [2048, 4096]
        # [b, p, rc, j0, j] -> [rc, p, j0, b, j]
        g9 = gs.reshape(NBLK, 128, NRC, 4, 128).transpose(2, 1, 3, 0, 4)
        m["g"] = np.ascontiguousarray(g9.reshape(NRC, 128, NBLK * 512))
        m["ohtdr"] = oht_all[c]
        in_maps.append(m)
    return in_maps


def kernel(**inputs):
    if "nc" not in _CACHE:
        _CACHE["nc"], _CACHE["in_names"] = _build()
    nc = _CACHE["nc"]
    in_maps = _prep_inputs(**inputs)
    res = bass_utils.run_bass_kernel_spmd(nc, in_maps, core_ids=list(range(NCORES)))
    return np.asarray(res.results[0]["out"], np.float32)


# revision 31
# speedup vs baseline: 1.6985x; 1.1025x over previous
import sys

for _p in ("/opt/trn_rl_repo",):
    if _p not in sys.path:
        sys.path.insert(0, _p)

import numpy as np
import ml_dtypes

import concourse.bass as bass
import concourse.bacc as bacc
import concourse.tile as tile
import concourse.mybir as mybir
from concourse import bass_utils

F32 = mybir.dt.float32
BF16 = mybir.dt.bfloat16
FP8 = mybir.dt.float8e4

NP_BF16 = ml_dtypes.bfloat16
NP_FP8 = ml_dtypes.float8_e4m3

EMBED = 512
MID = 512
FINAL = 1024
GLIMPSES = 2
NOBJ = 2048
NREL = 32768
NCORES = 8
RCH = NREL // NCORES          # 4096 relations per core
NRC = RCH // 512              # 8 relation chunks of 512 per core
NRB = RCH // 128              # 32 relation blocks of 128 per core
NBLK = NOBJ // 128            # 16 object blocks, no padding
NPAIR = NBLK // 2             # 8 object-block pairs (fp8 DoubleRow)
NE = 160                      # padded entity count (151 -> 160)
CSCALE = 65536.0              # fp8 range scaling for recip in wT
VSCALE = 1024.0               # fp8 range scaling for abc tables
CV = CSCALE * VSCALE
F2SCALE = 1.0                 # fc2 runs in bf16; no range scaling

# qpack (bf16) column offsets
QR0H = 0                      # qrelu0/CV rows 0:128
QR0L = QR0H + MID             # qrelu0/CV rows 128:160 (on partitions 0:32)
Z1SH = QR0L + MID             # z1 static/CV rows 0:128
Z1SL = Z1SH + MID             # z1 static/CV rows 128:160
IDENT = Z1SL + MID            # 128 identity
ONESC = IDENT + 128           # 1 column of ones
NQ = ONESC + 1

# lpack (bf16) column offsets: wfold/CV k-blocks, then Fpack (g, kb, jt)
WFOLD = 0                     # 4 * MID
FPK = WFOLD + 4 * MID         # 2 * 4 * 4 * 128 = 4096
NL = FPK + GLIMPSES * 16 * 128

# rowp (bf16) [1, x]: ones 0:128, fc2b*F2SCALE 128:1152
RONES = 0
RB32 = 128
NR = RB32 + FINAL

_CACHE = {}


def _wn(v, g):
    return (v * (g / np.linalg.norm(v.astype(np.float64)))).astype(np.float32)


def _build():
    """Builds the Bass program once. Returns (nc, input tensor names)."""
    nc = bacc.Bacc(
        "TRN2",
        target_bir_lowering=False,
        debug=False,
        enable_asserts=False,
        num_devices=NCORES,
    )

    # ---- DRAM I/O -------------------------------------------------------
    d_g = nc.dram_tensor("g", [NRC, 128, NBLK * 512], FP8, kind="ExternalInput")
    d_wT = nc.dram_tensor("wT", [128, NPAIR * 2 * NE], FP8, kind="ExternalInput")
    d_ohtdr = nc.dram_tensor("ohtdr", [77, 2 * RCH], FP8, kind="ExternalInput")
    d_abcdr = nc.dram_tensor("abcdr", [77, GLIMPSES * 1024], FP8,
                             kind="ExternalInput")
    d_qpack = nc.dram_tensor("qpack", [128, NQ], BF16, kind="ExternalInput")
    d_lpack = nc.dram_tensor("lpack", [128, NL], BF16, kind="ExternalInput")
    d_fc2dr = nc.dram_tensor("fc2dr", [128, 4 * FINAL], BF16,
                             kind="ExternalInput")
    d_bpack = nc.dram_tensor("bpack", [128, 13], F32, kind="ExternalInput")
    d_rowp = nc.dram_tensor("rowp", [1, NR], BF16, kind="ExternalInput")
    d_out = nc.dram_tensor("out", [1, FINAL], F32, kind="ExternalOutput")

    with tile.TileContext(nc) as tc:
        _emit(nc, tc, locals())

    nc.compile()
    in_names = ["g", "wT", "ohtdr", "abcdr", "qpack", "lpack", "fc2dr",
                "bpack", "rowp"]
    return nc, in_names


def _emit(nc, tc, d):
    AT = mybir.ActivationFunctionType
    OP = mybir.AluOpType
    rg = [list(range(NCORES))]

    with (
        tc.tile_pool(name="persist", bufs=1) as pp,
        tc.tile_pool(name="work", bufs=6) as wp,
        tc.tile_pool(name="gpool", bufs=5) as gp,
        tc.tile_pool(name="pvv", bufs=2, space="PSUM") as pvv,
        tc.tile_pool(name="pat", bufs=2, space="PSUM") as pat,
        tc.tile_pool(name="pbh", bufs=2, space="PSUM") as pbhp,
        tc.tile_pool(name="plo", bufs=2, space="PSUM") as plop,
        tc.tile_pool(name="dram", bufs=1, space="DRAM") as dp,
    ):
        # ---- input DMAs: one queue (SP), strict priority order ----------
        wT = pp.tile([128, NPAIR * 2 * NE], FP8, name="wT", tag="wT")
        nc.sync.dma_start(wT[:], d["d_wT"][:, :])
        ohtdr = pp.tile([77, 2 * RCH], FP8, name="ohtdr", tag="ohtdr")
        abcdr = pp.tile([77, GLIMPSES * 1024], FP8, name="abcdr", tag="abcdr")
        qpk = pp.tile([128, NQ], BF16, name="qpk", tag="qpk")
        g8rc = []
        for rc in range(NRC):
            t = gp.tile([128, NBLK * 512], FP8, name=f"g8rc{rc}", tag="g8")
            nc.sync.dma_start(t[:], d["d_g"][rc])
            g8rc.append(t)
            if rc == 0:
                nc.sync.dma_start(ohtdr[:], d["d_ohtdr"][:, :])
                nc.sync.dma_start(abcdr[:], d["d_abcdr"][:, :])
            elif rc == 1:
                nc.sync.dma_start(qpk[:], d["d_qpack"][:, :])

        lpk = pp.tile([128, NL], BF16, name="lpk", tag="lpk")
        nc.sync.dma_start(lpk[:], d["d_lpack"][:, :])
        fc2dr = pp.tile([128, 4 * FINAL], BF16, name="fc2dr", tag="fc2dr")
        nc.sync.dma_start(fc2dr[:], d["d_fc2dr"][:, :])
        bpack = pp.tile([128, 13], F32, name="bpack", tag="bpack")
        nc.sync.dma_start(bpack[:], d["d_bpack"][:, :])
        rowp = pp.tile([1, NR], BF16, name="rowp", tag="rowp")
        nc.sync.dma_start(rowp[:], d["d_rowp"][:, :])

        # ---- views into the packs ---------------------------------------
        qr0hi = qpk[:, QR0H:QR0H + MID]
        qr0lo = qpk[0:32, QR0L:QR0L + MID]
        z1shi = qpk[:, Z1SH:Z1SH + MID]
        z1slo = qpk[0:32, Z1SL:Z1SL + MID]
        ident = qpk[:, IDENT:IDENT + 128]
        onesc = qpk[:, ONESC:ONESC + 1]
        wfold = lpk[:, WFOLD:WFOLD + 4 * MID]
        fpk = lpk[:, FPK:FPK + GLIMPSES * 16 * 128]
        ones_row = rowp[0:1, RONES:RONES + 128]
        fc2b32 = rowp[0:1, RB32:RB32 + FINAL]
        baseT = bpack[:, 0:4]
        onesf = bpack[:, 4:5]
        fc2bT = bpack[:, 5:13]

        # ---- persistent state -------------------------------------------
        a2hi = pp.tile([128, NRB * 128], FP8, name="a2hi", tag="a2hi")
        a2lo = pp.tile([128, NRB * 32], FP8, name="a2lo", tag="a2lo")
        hbf = [pp.tile([128, 4], BF16, name=f"hbf{g}", tag=f"hbf{g}")
               for g in range(GLIMPSES)]
        qr1hi = pp.tile([128, MID], BF16, name="qr1hi", tag="qr1hi")
        qr1lo = pp.tile([32, MID], BF16, name="qr1lo", tag="qr1lo")
        zrow = pp.tile([1, MID], BF16, name="zrow", tag="zrow")
        o1f = pp.tile([128, 4], F32, name="o1f", tag="o1f")
        o1T8 = pp.tile([128, 4], BF16, name="o1T8", tag="o1T8")
        out_sb = pp.tile([128, 8], F32, name="out_sb", tag="out_sb")

        pbh = [pbhp.tile([128, 512], F32, name=f"pbh{g}", tag="pbh")
               for g in range(GLIMPSES)]
        pbl = [plop.tile([32, 512], F32, name=f"pbl{g}", tag="plo")
               for g in range(GLIMPSES)]

        # copy engines (GPSIMD cannot access PSUM, so only Act/DVE here)
        def relu_copy(eng_i, dst, src):
            if eng_i == 0:
                nc.scalar.activation(dst, src, AT.Relu)
            else:
                nc.vector.tensor_scalar(dst, src, 0.0, None, OP.max)

        def plain_copy(eng_i, dst, src):
            if eng_i == 0:
                nc.scalar.copy(dst, src)
            else:
                nc.vector.tensor_copy(dst, src)

        # ================= streaming phase ===============================
        # The stream carries only glimpse-0 work (A^T aggregation, v0, B0);
        # glimpse-1 v/B runs later, hidden under the first AllGather.
        # B matmuls for chunk rc are emitted during chunk rc+1 (PE executes
        # in-order; the one-chunk delay guarantees the v relu-copies feeding
        # them are long done, so PE never stalls on the vector engines).
        def emit_v(g, rb, dst, eng_i):
            vps = pvv.tile([128, 512], F32, name=f"vps{g}{rb}", tag="vps")
            nc.tensor.matmul(
                vps[:],
                ohtdr[:, rb * 256:(rb + 1) * 256].rearrange(
                    "p (k n) -> p k n", k=2),
                abcdr[:, g * 1024:(g + 1) * 1024].rearrange(
                    "p (k n) -> p k n", k=2),
                start=True, stop=True,
                perf_mode=mybir.MatmulPerfMode.DoubleRow)
            relu_copy(eng_i, dst, vps[:])

        def emit_B(g, pr, vpair_t):
            vpair = vpair_t[:].rearrange("p (k n) -> p k n", k=2)
            nc.tensor.matmul(
                pbh[g][:],
                a2hi[:, pr * 256:(pr + 1) * 256].rearrange(
                    "p (k n) -> p k n", k=2),
                vpair, start=(pr == 0), stop=(pr == NRB // 2 - 1),
                perf_mode=mybir.MatmulPerfMode.DoubleRow)
            nc.tensor.matmul(
                pbl[g][:],
                a2lo[:, pr * 64:(pr + 1) * 64].rearrange(
                    "p (k n) -> p k n", k=2),
                vpair, start=(pr == 0), stop=(pr == NRB // 2 - 1),
                perf_mode=mybir.MatmulPerfMode.DoubleRow)

        vchp = {}
        for rc in range(NRC):
            g9 = g8rc[rc]
            for j0 in range(4):
                rb = rc * 4 + j0
                psAT = pat.tile([128, NE], F32, name=f"psAT{rb}", tag="at")
                for pb in range(NPAIR):
                    nc.tensor.matmul(
                        psAT[:],
                        g9[:, j0 * 2048 + pb * 256:
                           j0 * 2048 + (pb + 1) * 256].rearrange(
                            "p (k n) -> p k n", k=2),
                        wT[:, pb * 2 * NE:(pb + 1) * 2 * NE].rearrange(
                            "p (k n) -> p k n", k=2),
                        start=(pb == 0), stop=(pb == NPAIR - 1),
                        perf_mode=mybir.MatmulPerfMode.DoubleRow)
                plain_copy(1, a2hi[:, rb * 128:(rb + 1) * 128],
                           psAT[:, 0:128])
                plain_copy(1, a2lo[:, rb * 32:(rb + 1) * 32],
                           psAT[:, 128:160])
                pr = rb // 2
                half = rb % 2
                if half == 0:
                    vchp[pr] = pp.tile([128, 1024], FP8, name=f"vch0_{pr}",
                                       tag=f"vch0_{pr}")
                emit_v(0, rb, vchp[pr][:, half * 512:(half + 1) * 512], 0)
        # B0 burst: all inputs (a2, vch0) are ready by the time the last
        # chunk's copies land, so this runs back-to-back on the PE
        for pr in range(NRB // 2):
            emit_B(0, pr, vchp[pr])

        # ---- h partial for glimpse g (scales folded on host) -------------
        def emit_h(g, qhi, qlo):
            plo_t = wp.tile([32, 512], BF16, name=f"plo{g}", tag="plo")
            nc.vector.tensor_tensor(plo_t[:], pbl[g][:], qlo, OP.mult)
            phi = wp.tile([128, 512], BF16, name=f"phi{g}", tag="phi")
            nc.vector.tensor_tensor(phi[:], pbh[g][:], qhi, OP.mult)
            hps = pat.tile([1, MID], F32, name=f"hps{g}", tag="at")
            nc.tensor.matmul(hps[0:1, :], onesc[0:32, 0:1], plo_t[:],
                             start=True, stop=False)
            nc.tensor.matmul(hps[0:1, :], onesc[:, 0:1], phi[:],
                             start=False, stop=True)
            hsb = wp.tile([1, MID], F32, name=f"hsb{g}", tag="hsb")
            nc.scalar.copy(hsb[:], hps[0:1, :])
            h_in = dp.tile([1, MID], F32, name=f"h_in{g}", tag=f"h_in{g}")
            h_out = dp.tile([NCORES, MID], F32, name=f"h_out{g}",
                            tag=f"h_out{g}", addr_space="Shared")
            nc.sync.dma_start(h_in[:], hsb[:])
            nc.gpsimd.collective_compute(
                "AllGather", OP.bypass, replica_groups=rg,
                ins=[h_in[:].opt()], outs=[h_out[:].opt()])
            return h_out

        # Gathered [8, 512] -> natural-layout DMA, then 4 matmul-reduces
        # against a ones column -> hT in [128, 4] partition layout.
        def emit_h_sum(g, h_out):
            hgn = wp.tile([NCORES, MID], F32, name=f"hgn{g}", tag="hgn")
            nc.sync.dma_start(hgn[:], h_out[:, :])
            htp = pat.tile([128, 4], F32, name=f"htp{g}", tag="at")
            for c in range(4):
                nc.tensor.matmul(htp[:, c:c + 1],
                                 hgn[:, c * 128:(c + 1) * 128],
                                 onesf[0:NCORES, :], start=True, stop=True)
            nc.scalar.copy(hbf[g][:], htp[:])

        h_out0 = emit_h(0, qr0hi, qr0lo)

        # glimpse-1 v/B, hidden under the first AllGather's latency.
        # Copies go to Act/DVE only: the Pool queue head is blocked by the
        # collective_compute instruction until the AllGather completes.
        for pr in range(NRB // 2):
            vt = wp.tile([128, 1024], FP8, name=f"vch1_{pr}", tag="vch1")
            emit_v(1, 2 * pr, vt[:, 0:512], 0)
            emit_v(1, 2 * pr + 1, vt[:, 512:1024], 1)
            emit_B(1, pr, vt)

        # keep the PE p-state warm through the tail of the AllGather so the
        # mid-section matmuls run at full clock (no data dependencies)
        for w in range(20):
            wps = pvv.tile([1, MID], F32, name=f"warm{w}", tag="vps")
            nc.tensor.matmul(wps[:], ones_row[0:1, 0:1], zrow[0:1, :],
                             start=True, stop=True)

        emit_h_sum(0, h_out0)

        # ---- z1 row = h0 @ (wfold/CV) ------------------------------------
        zps = pat.tile([1, MID], F32, name="zps", tag="at")
        for kb in range(4):
            nc.tensor.matmul(zps[:], hbf[0][:, kb:kb + 1],
                             wfold[:, kb * MID:(kb + 1) * MID],
                             start=(kb == 0), stop=(kb == 3))
        nc.vector.tensor_copy(zrow[:], zps[:])

        # qrelu1 = relu(z1static + 1 (x) zrow), hi/lo via identity matmuls
        zbh = pvv.tile([128, MID], F32, name="zbh", tag="vps")
        nc.tensor.matmul(zbh[:], ident[:], z1shi, start=True, stop=False)
        nc.tensor.matmul(zbh[:], ones_row[0:1, 0:128], zrow[:],
                         start=False, stop=True)
        nc.scalar.activation(qr1hi[:], zbh[:], AT.Relu)
        zbl = pvv.tile([32, MID], F32, name="zbl", tag="vps")
        nc.tensor.matmul(zbl[:], ident[0:32, 0:32], z1slo,
                         start=True, stop=False)
        nc.tensor.matmul(zbl[:], ones_row[0:1, 0:32], zrow[:],
                         start=False, stop=True)
        nc.vector.tensor_scalar(qr1lo[:], zbl[:], 0.0, None, OP.max)

        h_out1 = emit_h(1, qr1hi[:], qr1lo[:])

        # ---- tail: o1 = relu(base + h0@F0 + h1@F1) ------------------------
        # h0@F0 runs under the second AllGather.
        o1ps = pvv.tile([128, 4], F32, name="o1ps", tag="vps")
        for jt in range(4):
            for kb in range(4):
                nc.tensor.matmul(
                    o1ps[:, jt:jt + 1],
                    fpk[:, (0 * 4 + kb) * 512 + jt * 128:
                        (0 * 4 + kb) * 512 + (jt + 1) * 128],
                    hbf[0][:, kb:kb + 1],
                    start=(kb == 0), stop=False)

        emit_h_sum(1, h_out1)
        for jt in range(4):
            for kb in range(4):
                nc.tensor.matmul(
                    o1ps[:, jt:jt + 1],
                    fpk[:, (1 * 4 + kb) * 512 + jt * 128:
                        (1 * 4 + kb) * 512 + (jt + 1) * 128],
                    hbf[1][:, kb:kb + 1],
                    start=False, stop=(kb == 3))
        nc.vector.tensor_tensor(o1f[:], o1ps[:], baseT, OP.add)
        nc.vector.tensor_scalar(o1T8[:], o1f[:], 0.0, None, OP.max)

        # fc2 in partition layout: out2T[p, nt] column accumulations via
        # N=1 matmuls (cost is free-dim based, so these are pstate-immune),
        # then bias+relu on DVE and a strided DMA back to row layout.
        o2ps = pat.tile([128, 8], F32, name="o2ps", tag="at")
        for nt in range(8):
            for kb in range(4):
                nc.tensor.matmul(
                    o2ps[:, nt:nt + 1],
                    fc2dr[:, (kb * 8 + nt) * 128:(kb * 8 + nt) * 128 + 128],
                    o1T8[:, kb:kb + 1], start=(kb == 0), stop=(kb == 3))
        nc.vector.tensor_tensor(out_sb[:], o2ps[:], fc2bT, OP.add)
        nc.vector.tensor_scalar(out_sb[:], out_sb[:], 0.0, None, OP.max)
        nc.sync.dma_start(
            d["d_out"][0:1, :].rearrange("a (c p) -> p (a c)", p=128),
            out_sb[:])


def _prep_inputs(entities, relations, graph, obj_tab, head_tab, tail_tab, pred_tab,
                 lin_v_v, lin_v_g, lin_v_b, lin_q_v, lin_q_g, lin_q_b,
                 lin_a_v, lin_a_g, lin_a_b, fc1_w, fc1_b, fc2_w, fc2_b):
    ent = np.asarray(entities).astype(np.int64)
    rel = np.asarray(relations).astype(np.int64)
    graph = np.asarray(graph, dtype=np.float32)

    abc = np.zeros((GLIMPSES, 153, MID), np.float32)
    wqT = np.zeros((GLIMPSES, EMBED, MID), np.float32)
    waT = np.zeros((GLIMPSES, MID, MID), np.float32)
    ba = np.zeros((GLIMPSES, MID), np.float32)
    bq = np.zeros((GLIMPSES, MID), np.float32)
    for g in range(GLIMPSES):
        Wv = _wn(np.asarray(lin_v_v[g], np.float32), float(lin_v_g[g]))
        abc[g, 0:51] = head_tab[:51] @ Wv[:, 0:EMBED].T + np.asarray(lin_v_b[g], np.float32)
        abc[g, 51:102] = tail_tab[:51] @ Wv[:, EMBED:2 * EMBED].T
        abc[g, 102:153] = pred_tab[:51] @ Wv[:, 2 * EMBED:3 * EMBED].T
        Wq = _wn(np.asarray(lin_q_v[g], np.float32), float(lin_q_g[g]))
        wqT[g] = Wq.T
        bq[g] = np.asarray(lin_q_b[g], np.float32)
        Wa = _wn(np.asarray(lin_a_v[g], np.float32), float(lin_a_g[g]))
        waT[g] = Wa.T
        ba[g] = np.asarray(lin_a_b[g], np.float32)

    obj_tab = np.asarray(obj_tab, np.float32)
    fc1_b = np.asarray(fc1_b, np.float32)
    fc2_b = np.asarray(fc2_b, np.float32)
    fc1T = np.ascontiguousarray(fc1_w.astype(np.float32).T)   # [512, 512]
    fc2T = np.ascontiguousarray(fc2_w.astype(np.float32).T)   # [512, 1024]

    # relation one-hots (per core), fp8 DoubleRow layout [77, rb*256 + ko*128 + j]
    oht_all = []
    ar = np.arange(RCH)
    for c in range(NCORES):
        rc_ = rel[c * RCH:(c + 1) * RCH]
        m = np.zeros((154, RCH), np.float32)
        m[rc_[:, 0], ar] = 1.0
        m[rc_[:, 1] + 51, ar] = 1.0
        m[rc_[:, 2] + 102, ar] = 1.0
        ohtdr = m.reshape(2, 77, NRB, 128).transpose(1, 2, 0, 3)
        oht_all.append(np.ascontiguousarray(
            ohtdr.reshape(77, 2 * RCH)).astype(NP_FP8))

    # abc tables, fp8 DoubleRow layout [77, g*1024 + ko*512 + m]
    abc_pad = np.zeros((154, GLIMPSES, MID), np.float32)
    abc_pad[0:153] = (abc * VSCALE).transpose(1, 0, 2)
    abcdr = abc_pad.reshape(2, 77, GLIMPSES, MID).transpose(1, 2, 0, 3)
    abcdr = np.ascontiguousarray(abcdr.reshape(77, GLIMPSES * 1024)).astype(NP_FP8)

    # wT: scatter matrix [obj, entity] with recip scaling, DoubleRow layout
    recip = (CSCALE / (graph.sum(axis=1, dtype=np.float32) + 1e-9)).astype(np.float32)
    wTf = np.zeros((NOBJ, NE), np.float32)
    wTf[np.arange(NOBJ), ent] = recip
    wTdr = wTf.reshape(NPAIR, 2, 128, NE).transpose(2, 0, 1, 3)
    wTdr = np.ascontiguousarray(wTdr.reshape(128, NPAIR * 2 * NE)).astype(NP_FP8)

    # per-entity tables, scaled by 1/CV so device h comes out unscaled
    qr0 = np.zeros((NE, MID), np.float32)
    qr0[0:151] = np.maximum(obj_tab @ wqT[0] + bq[0], 0.0) * (1.0 / CV)
    z1s = np.zeros((NE, MID), np.float32)
    z1s[0:151] = (obj_tab @ wqT[1] + (ba[0] @ wqT[1] + bq[1])) * (1.0 / CV)

    qpack = np.zeros((128, NQ), np.float32)
    qpack[:, QR0H:QR0H + MID] = qr0[0:128]
    qpack[0:32, QR0L:QR0L + MID] = qr0[128:160]
    qpack[:, Z1SH:Z1SH + MID] = z1s[0:128]
    qpack[0:32, Z1SL:Z1SL + MID] = z1s[128:160]
    qpack[:, IDENT:IDENT + 128] = np.eye(128, dtype=np.float32)
    qpack[:, ONESC:ONESC + 1] = 1.0

    # tail folding: sgq0 = sum over objects of q0 rows
    cnt = np.bincount(ent, minlength=151).astype(np.float32)
    sgq0 = cnt @ obj_tab
    base = (sgq0 + NOBJ * (ba[0] + ba[1])) @ fc1T + fc1_b    # [512]
    bpack = np.zeros((128, 13), np.float32)
    bpack[:, 0:4] = base.reshape(4, 128).T
    bpack[:, 4] = 1.0
    bpack[:, 5:13] = fc2_b.reshape(8, 128).T
    F = [NOBJ * (waT[g] @ fc1T) for g in range(GLIMPSES)]    # [512, 512] each

    lpack = np.zeros((128, NL), np.float32)
    wfold = (waT[0] @ wqT[1]) * (1.0 / CV)
    lpack[:, WFOLD:WFOLD + 4 * MID] = np.concatenate(
        [wfold[kb * 128:(kb + 1) * 128, :] for kb in range(4)], axis=1)
    fblocks = []
    for g in range(GLIMPSES):
        for kb in range(4):
            for jt in range(4):
                fblocks.append(F[g][kb * 128:(kb + 1) * 128,
                                    jt * 128:(jt + 1) * 128])
    lpack[:, FPK:FPK + GLIMPSES * 16 * 128] = np.concatenate(fblocks, axis=1)

    # fc2 weights, partition-block layout [128, (kb*8 + nt)*128 + n]
    fc2s = fc2T.reshape(4, 128, 8, 128).transpose(1, 0, 2, 3)
    fc2dr = np.ascontiguousarray(
        fc2s.reshape(128, 4 * FINAL)).astype(NP_BF16)

    rowp = np.zeros((1, NR), np.float32)
    rowp[0, RONES:RONES + 128] = 1.0

    base_map = {
        "wT": wTdr,
        "abcdr": abcdr,
        "qpack": qpack.astype(NP_BF16),
        "lpack": lpack.astype(NP_BF16),
        "fc2dr": fc2dr,
        "bpack": bpack,
        "rowp": rowp.astype(NP_BF16),
    }
    in_maps = []
    for c in range(NCORES):
        m = dict(base_map)
        gs = graph[:, c * RCH:(c + 1) * RCH].astype(NP_FP8)   # [2048, 4096]
        # [b, p, rc, j0, j] -> [rc, p, j0, b, j]
        g9 = gs.reshape(NBLK, 128, NRC, 4, 128).transpose(2, 1, 3, 0, 4)
        m["g"] = np.ascontiguousarray(g9.reshape(NRC, 128, NBLK * 512))
        m["ohtdr"] = oht_all[c]
        in_maps.append(m)
    return in_maps


def kernel(**inputs):
    if "nc" not in _CACHE:
        _CACHE["nc"], _CACHE["in_names"] = _build()
    nc = _CACHE["nc"]
    in_maps = _prep_inputs(**inputs)
    res = bass_utils.run_bass_kernel_spmd(nc, in_maps, core_ids=list(range(NCORES)))
    return np.asarray(res.results[0]["out"], np.float32)
